# revision 31
# baseline (speedup 1.0000x reference)
"""DLRM on 8 Trainium2 NeuronCores, data-parallel over the batch.  v7.

v7 = staged phase-I pipeline, two btiles deep (trace-driven):
  - phase I is emitted as five independent stages (gather / zt-transpose /
    gram / extract / x1-transpose) so no engine queue waits on work that
    was produced later than its own next item (head-of-line blocking).
  - btile v's MLP overlaps phase I of btile v+2 (x1ta triple-buffered);
    the copies->extract->x1t tail gets a ~2-btile window, absorbing the
    8-sem-lane round-robin coupling between unrelated DMAs.
  - extracts all on the scalar ring: on gpsimd they poison the per-btile
    SWDGE DRAIN barrier (9.9us stalls); sync stays transposes-only.
  - single wide DMA per weight matrix (3D AP over k-tiles), issued on
    sync at kernel start; w1f host-padded to 768 rows.
  - x1 transpose lands directly in its x1ta slot (strided 3D dst AP).
Earlier: MLP ReLU+bias on DVE (tensor_scalar add+max); bf16 table;
both xbar transposes serialized on sync (concurrent DMA-transposes on
the two HWDGE rings corrupt data); ACT/DVE alternate the PSUM copies.
"""

import numpy as np
import ml_dtypes

import concourse.bass as bass
import concourse.mybir as mybir
import concourse.tile as tile
from concourse import bacc
from concourse.bass import ds, ts

F32 = mybir.dt.float32
BF16 = mybir.dt.bfloat16
F8 = mybir.dt.float8e4
I32 = mybir.dt.int32
DR = mybir.MatmulPerfMode.DoubleRow

N_CAT = 26
N_FEAT = 27          # 26 embeddings + dense
E = 128
P = 128
N_CORES = 8

AFT = mybir.ActivationFunctionType


class Cfg:
    def __init__(self, vocab=100000, nblk=16):
        self.vocab = vocab
        self.nblk = nblk                  # 128-sample blocks per core
        self.bc = nblk * P                # batch per core
        self.vblk = min(4, nblk)          # blocks per btile
        self.vt = self.vblk * P           # btile size (free dim N)
        self.nv = nblk // self.vblk       # btiles per core


def build_kernel(nc: bass.Bass, tc: tile.TileContext, cfg: Cfg):
    from contextlib import ExitStack
    with ExitStack() as ctx:
        _build_kernel(nc, tc, cfg, ctx)


def _build_kernel(nc: bass.Bass, tc: tile.TileContext, cfg: Cfg, ctx):
    NBLK, BC, VT, NV, VBLK = cfg.nblk, cfg.bc, cfg.vt, cfg.nv, cfg.vblk

    # ---------------- DRAM I/O ----------------
    emb = nc.dram_tensor("emb", [N_CAT * cfg.vocab, E], BF16, kind="ExternalInput").ap()
    # idx[p, t*27 + c] = emb row for block t, flat n = c*128 + p, n = s*27 + j
    idx_d = nc.dram_tensor("idx", [P, NBLK * N_FEAT], I32, kind="ExternalInput").ap()
    xt_d = nc.dram_tensor("xt", [13, BC], BF16, kind="ExternalInput").ap()

    def win(name, shape):
        return nc.dram_tensor(name, shape, BF16, kind="ExternalInput").ap()

    wd1_d = win("wd1", [13, 512])
    wd2_d = win("wd2", [512, 256])
    wdf_d = win("wdf", [256, 128])
    w1f_d = win("w1f", [768, 1024])
    w1d_d = win("w1d", [128, 1024])
    # fp8 DoubleRow-interleaved (x16 scaled): [p, (j, i, m)] with
    # w[p, j, i, m] = 16 * W[j*256 + i*128 + p, m]
    wt2_d = nc.dram_tensor("wt2", [P, 8 * 1024], F8, kind="ExternalInput").ap()
    wt3_d = nc.dram_tensor("wt3", [P, 8 * 512], F8, kind="ExternalInput").ap()
    wt4_d = nc.dram_tensor("wt4", [P, 4 * 256], F8, kind="ExternalInput").ap()
    wo_d = win("wo", [256, 1])

    def bin_(name, shape):
        return nc.dram_tensor(name, shape, F32, kind="ExternalInput").ap()

    bd1_d = bin_("bd1", [P, 4])
    bd2_d = bin_("bd2", [P, 2])
    bdf_d = bin_("bdf", [P, 1])
    bt1_d = bin_("bt1", [P, 8])
    bt2_d = bin_("bt2", [P, 8])
    bt3_d = bin_("bt3", [P, 4])
    bt4_d = bin_("bt4", [P, 2])
    bo_d = bin_("bo", [1, 1])

    out_d = nc.dram_tensor("out", [BC, 1], F32, kind="ExternalOutput").ap()

    x1d = [
        nc.dram_tensor(f"x1scratch{t}", [P, 768], BF16, kind="Internal").ap()
        for t in range(NBLK)
    ]

    # ---------------- pools ----------------
    const = ctx.enter_context(tc.tile_pool(name="const", bufs=1))
    gpool = ctx.enter_context(tc.tile_pool(name="gather", bufs=5))
    zpool = ctx.enter_context(tc.tile_pool(name="zt", bufs=4))
    spool = ctx.enter_context(tc.tile_pool(name="spack", bufs=4))
    bpool = ctx.enter_context(tc.tile_pool(name="x1b", bufs=2))
    xtpool = ctx.enter_context(tc.tile_pool(name="x1t", bufs=3))
    opool = ctx.enter_context(tc.tile_pool(name="acts", bufs=1))
    p_s = ctx.enter_context(tc.tile_pool(name="pint", bufs=3, space="PSUM"))
    p_m = ctx.enter_context(tc.tile_pool(name="pmlp", bufs=3, space="PSUM"))


    ALU = mybir.AluOpType

    def relu_b(out, pm, bias_ap):
        nc.vector.tensor_scalar(
            out=out, in0=pm, scalar1=bias_ap, scalar2=0.0,
            op0=ALU.add, op1=ALU.max,
        )

    # ---------------- load constants ----------------
    # startup-critical first (bottom MLP inputs), big weights in single
    # wide DMAs on the sync ring (cheap descriptor gen, early issue)
    idx_sb = const.tile([P, NBLK * N_FEAT], I32)
    nc.scalar.dma_start(out=idx_sb[:], in_=idx_d)
    xtb = const.tile([13, BC], BF16, name="xtb")
    nc.scalar.dma_start(out=xtb[:], in_=xt_d)

    def load_w(name, d, k, n, eng):
        """One DMA for the whole weight; returns per-128-row k-tile views."""
        if k <= P:
            t = const.tile([k, n], BF16, name=name)
            eng.dma_start(out=t[:], in_=d[0:k, :])
            return [t[:]]
        T = d.shape[0] // P
        big = const.tile([P, T * n], BF16, name=name)
        eng.dma_start(
            out=big[:].rearrange("p (t n) -> p t n", t=T),
            in_=d.rearrange("(t p) n -> p t n", p=P),
        )
        views = []
        for i in range(T):
            ck = min(P, k - i * P)
            if ck <= 0:
                break
            views.append(big[ds(0, ck), ds(i * n, n)])
        return views

    def load_b(name, d, nm):
        t = const.tile([d.shape[0], nm], F32, name=name)
        nc.scalar.dma_start(out=t[:], in_=d)
        return t

    wd1 = load_w("wd1", wd1_d, 13, 512, nc.scalar)
    wd2 = load_w("wd2", wd2_d, 512, 256, nc.scalar)
    wdf = load_w("wdf", wdf_d, 256, 128, nc.scalar)
    w1f = load_w("w1f", w1f_d, 729, 1024, nc.sync)
    w1d = load_w("w1d", w1d_d, 128, 1024, nc.sync)

    def load_w8(name, d):
        t = const.tile(list(d.shape), F8, name=name)
        nc.sync.dma_start(out=t[:], in_=d)
        return t

    wt2 = load_w8("wt2", wt2_d)
    wt3 = load_w8("wt3", wt3_d)
    wt4 = load_w8("wt4", wt4_d)
    wo = load_w("wo", wo_d, 256, 1, nc.sync)

    def dr_mm(pm, wq, o_in, j, nj, m, n_out):
        """One DoubleRow fp8 matmul: contraction k-pair j, m-tile m."""
        lhsT = wq[:, ds(j * 2 * n_out, 2 * n_out)].rearrange(
            "p (i mm) -> p i mm", i=2
        )[:, :, ds(m * P, P)]
        rhs = o_in[:, ds(2 * j * VT, 2 * VT)].rearrange("p (i n) -> p i n", i=2)
        nc.tensor.matmul(
            pm[:], lhsT, rhs, start=(j == 0), stop=(j == nj - 1), perf_mode=DR
        )
    bd1 = load_b("bd1", bd1_d, 4)
    bd2 = load_b("bd2", bd2_d, 2)
    bdf = load_b("bdf", bdf_d, 1)
    bt1 = load_b("bt1", bt1_d, 8)
    bt2 = load_b("bt2", bt2_d, 8)
    bt3 = load_b("bt3", bt3_d, 4)
    bt4 = load_b("bt4", bt4_d, 2)
    bo = load_b("bo", bo_d, 1)

    # ---------------- bottom MLP (whole core batch) ----------------
    h1 = [const.tile([P, BC], BF16, name=f"h1_{m}") for m in range(4)]
    for m in range(4):
        for v in range(NV):
            pm = p_m.tile([P, VT], F32)
            nc.tensor.matmul(
                pm[:], wd1[0][:, ts(m, P)], xtb[:, ts(v, VT)],
                start=True, stop=True,
            )
            relu_b(h1[m][:, ts(v, VT)], pm[:], bd1[:, m : m + 1])

    h2 = [const.tile([P, BC], BF16, name=f"h2_{m}") for m in range(2)]
    for m in range(2):
        for v in range(NV):
            pm = p_m.tile([P, VT], F32)
            for k in range(4):
                nc.tensor.matmul(
                    pm[:], wd2[k][:, ts(m, P)], h1[k][:, ts(v, VT)],
                    start=(k == 0), stop=(k == 3),
                )
            relu_b(h2[m][:, ts(v, VT)], pm[:], bd2[:, m : m + 1])

    denseT = const.tile([P, BC], BF16)
    for v in range(NV):
        pm = p_m.tile([P, VT], F32)
        for k in range(2):
            nc.tensor.matmul(
                pm[:], wdf[k][:], h2[k][:, ts(v, VT)],
                start=(k == 0), stop=(k == 1),
            )
        relu_b(denseT[:, ts(v, VT)], pm[:], bdf[:, 0:1])

    # ---------------- phase I, split into independently-emitted stages --
    # Per-engine queues execute in (priority = emission) order, so a stage
    # that waits on cross-engine work must NOT be emitted ahead of work the
    # same engine needs soon (head-of-line blocking).  Stages for block t:
    #   gather(t)   gpsimd   indirect gather          -> g[t]
    #   zt(t)       sync     xbar transpose + ztd     -> zt[t]
    #   gram(t)     PE       32 Gram MMs + 8 copies   -> spk[t]
    #   ext(t)      scalar/gpsimd  4 pair-extract DMAs -> x1d[t] (DRAM)
    #   x1t(t)      sync     xbar transpose           -> x1ta slot
    x1tas = {}
    g_t, zt_t, spk_t = {}, {}, {}

    def get_x1ta(v):
        if v not in x1tas:
            x1tas[v] = xtpool.tile([P, 6 * VBLK * P], BF16, name="x1ta")
        return x1tas[v]

    def stage_gather(t):
        if t >= NBLK:
            return
        g = gpool.tile([P, N_FEAT * E], BF16)
        g_t[t] = g
        nc.gpsimd.indirect_dma_start(
            out=g[:],
            out_offset=None,
            in_=emb,
            in_offset=bass.IndirectOffsetOnAxis(
                ap=idx_sb[:, t * N_FEAT : (t + 1) * N_FEAT], axis=0
            ),
        )

    def stage_zt(t):
        if t >= NBLK:
            return
        # Both xbar transposes stay on ONE ring: concurrent DMA-transposes
        # on the two HWDGE rings corrupt data (xbar hazard; Tile does not
        # guard it). Serializing them on sync keeps results correct.
        zt = zpool.tile([P, N_FEAT * E], BF16)
        zt_t[t] = zt
        nc.sync.dma_start(
            out=zt[:].rearrange("p (c s) -> p c s", c=N_FEAT),
            in_=g_t.pop(t)[:],
            transpose=True,
        )
        ztd = zt[:].rearrange("p (s j) -> p s j", j=N_FEAT)[:, :, N_CAT]
        nc.vector.tensor_copy(out=ztd, in_=denseT[:, ts(t, P)])

    def stage_gram(t):
        if t >= NBLK:
            return
        zt = zt_t.pop(t)
        spk = spool.tile([108, 32 * 108], BF16)
        spk_t[t] = spk
        for q0 in range(0, 32, 4):
            pi = p_s.tile([108, 4 * 108], F32)
            for q in range(q0, q0 + 4):
                op = zt[:, ds(q * 108, 108)]
                nc.tensor.matmul(
                    pi[:, ts(q - q0, 108)], op, op, start=True, stop=True
                )
            if (q0 // 4) % 2 == 0:
                nc.scalar.activation(
                    spk[:, ds(q0 * 108, 4 * 108)], pi[:], AFT.Copy
                )
            else:
                nc.vector.tensor_copy(
                    out=spk[:, ds(q0 * 108, 4 * 108)], in_=pi[:]
                )

    def stage_ext(t):
        if t >= NBLK:
            return
        spk = spk_t.pop(t)
        for sj in range(4):
            src = spk[ds(sj * 27, 27), :].rearrange(
                "p (q s2 j2) -> p q s2 j2", s2=4, j2=27
            )[:, :, sj, :]
            dstv = x1d[t][:, :729].rearrange(
                "(q s) (j1 j2) -> s j1 q j2", s=4, j2=27
            )[sj, :, :, :]
            nc.scalar.dma_start(out=dstv, in_=src)

    def stage_x1t(t):
        if t >= NBLK:
            return
        v, tt = t // VBLK, t % VBLK
        x1v = get_x1ta(v)[:].rearrange("p (k b s) -> p k b s", k=6, s=P)
        nc.sync.dma_start(
            out=x1v[:, :, tt, :],
            in_=x1d[t], transpose=True,
        )

    # ---------------- main: prologue + software-pipelined btiles --------
    # ~2.5-btile-deep pipeline: btile v runs its MLP while group v+2 moves
    # through zt/gram/ext/x1t and group v+3's gathers are issued.  Sync
    # runs the 4 zts first thing each btile (gathers landed a btile ago),
    # so gram slots in L2 always find their zt done.
    NPRO = min(2 * VBLK, NBLK)
    for t in range(NPRO):
        stage_gather(t)
    for t in range(NPRO):
        stage_zt(t)
        stage_gram(t)
        if t >= 2:
            stage_ext(t - 2)
    for t in range(max(0, NPRO - 2), NPRO):
        stage_ext(t)
    for t in range(NPRO, NPRO + VBLK):
        stage_gather(t)
    for t in range(NPRO):
        stage_x1t(t)

    for v in range(NV):
        x1ta = get_x1ta(v)
        x1t = [x1ta[:, ds(k * VT, VT)] for k in range(6)]
        b = (v + 2) * VBLK  # btile v+2's blocks, prepared during this one
        bn = (v + 3) * VBLK  # btile v+3's gathers, issued late this btile

        # layer 1: K = 729 pairs (+pad, zero weights) + 128 dense (bf16)
        o1 = opool.tile([P, 8 * VT], F8, name="o1")
        for m in range(8):
            pm = p_m.tile([P, VT], F32)
            for k in range(6):
                nc.tensor.matmul(
                    pm[:], w1f[k][:, ts(m, P)], x1t[k][: w1f[k].shape[0], :],
                    start=(k == 0), stop=False,
                )
            nc.tensor.matmul(
                pm[:], w1d[0][:, ts(m, P)], denseT[:, ts(v, VT)],
                start=False, stop=True,
            )
            relu_b(o1[:, ts(m, VT)], pm[:], bt1[:, m : m + 1])
            if m % 2 == 0:
                stage_zt(b + m // 2)
        x1tas.pop(v)

        # layers 2-4: fp8 DoubleRow, weights x16.  o2 holds 16*o2 (DVE
        # relu with x16 bias); L3 descales by 256 on ACT; o4 holds 16*o4.
        o2 = opool.tile([P, 8 * VT], F8, name="o2")
        for m in range(8):
            pm = p_m.tile([P, VT], F32)
            for j in range(4):
                dr_mm(pm, wt2, o1, j, 4, m, 1024)
            relu_b(o2[:, ts(m, VT)], pm[:], bt2[:, m : m + 1])
            if m % 2 == 1:
                stage_gram(b + m // 2)

        o3 = opool.tile([P, 4 * VT], F8, name="o3")
        for m in range(4):
            pm = p_m.tile([P, VT], F32)
            for j in range(4):
                dr_mm(pm, wt3, o2, j, 4, m, 512)
            nc.scalar.activation(
                o3[:, ts(m, VT)], pm[:], AFT.Relu,
                bias=bt3[:, m : m + 1], scale=1.0 / 256,
            )
            stage_ext(b + m)
            if m >= 1:
                stage_gather(bn + m - 1)

        o4 = opool.tile([P, 2 * VT], BF16, name="o4")
        for m in range(2):
            pm = p_m.tile([P, VT], F32)
            for j in range(2):
                dr_mm(pm, wt4, o3, j, 2, m, 256)
            relu_b(o4[:, ts(m, VT)], pm[:], bt4[:, m : m + 1])
            if m == 0:
                stage_x1t(b + 0)
                stage_gather(bn + 3)
            else:
                stage_x1t(b + 1)

        pm = p_m.tile([1, VT], F32)
        for k in range(2):
            nc.tensor.matmul(
                pm[:], wo[k][:], o4[:, ts(k, VT)],
                start=(k == 0), stop=(k == 1),
            )
        zf = bpool.tile([1, VT], F32, name="zfinal")
        nc.scalar.activation(zf[:], pm[:], AFT.Sigmoid, bias=bo[:], scale=1.0 / 16)
        nc.scalar.dma_start(
            out=out_d[ts(v, VT), :].rearrange("b one -> one b"), in_=zf[:]
        )
        stage_x1t(b + 2)
        stage_x1t(b + 3)


# ---------------------------------------------------------------------------
# host side
# ---------------------------------------------------------------------------

_CACHE = {}


def _get_nc(cfg: Cfg):
    key = (cfg.vocab, cfg.nblk)
    if key in _CACHE:
        return _CACHE[key]
    nc = bacc.Bacc(
        "TRN2",
        target_bir_lowering=False,
        debug=False,
        enable_asserts=False,
        num_devices=N_CORES,
    )
    with tile.TileContext(nc) as tc:
        build_kernel(nc, tc, cfg)
    nc.compile()
    _CACHE[key] = nc
    return nc


def _prep_host(inputs, cfg: Cfg):
    """Build the per-core in_maps from full inputs."""
    bf = ml_dtypes.bfloat16
    emb = np.ascontiguousarray(
        np.asarray(inputs["emb_table"], dtype=np.float32)
        .reshape(N_CAT * cfg.vocab, E)
        .astype(bf)
    )
    cat = np.asarray(inputs["cat_idx"])
    dx = np.asarray(inputs["dense_x"], dtype=np.float32)

    iu, ju = np.triu_indices(N_FEAT, k=1)
    wt1 = np.asarray(inputs["Wt1"], dtype=np.float32)  # [479, 1024]
    w1f = np.zeros((768, 1024), dtype=np.float32)  # 729 pairs padded to 6*128
    w1fv = w1f[:729].reshape(N_FEAT, N_FEAT, 1024)
    w1fv[iu, ju] = 0.5 * wt1[: len(iu)]
    w1fv[ju, iu] = 0.5 * wt1[: len(iu)]
    w1d = wt1[len(iu) :]  # [128, 1024]

    def b2(x, nm):  # bias [N] -> [128, nm]
        return np.ascontiguousarray(
            np.asarray(x, np.float32).reshape(nm, P).T
        )

    f8 = ml_dtypes.float8_e4m3

    def dr16(w, nj, n):
        """fp8 DoubleRow interleave, x16: [nj*256, n] -> [128, nj*2*n]."""
        a = (np.asarray(w, np.float32) * 16).reshape(nj, 2, P, n)
        return np.ascontiguousarray(
            a.transpose(2, 0, 1, 3).reshape(P, nj * 2 * n)
        ).astype(f8)

    shared = dict(
        emb=emb,
        wd1=np.asarray(inputs["Wd1"], bf),
        wd2=np.asarray(inputs["Wd2"], bf),
        wdf=np.asarray(inputs["Wdf"], bf),
        w1f=np.asarray(w1f, bf),
        w1d=np.ascontiguousarray(np.asarray(w1d, bf)),
        wt2=dr16(inputs["Wt2"], 4, 1024),
        wt3=dr16(inputs["Wt3"], 4, 512),
        wt4=dr16(inputs["Wt4"], 2, 256),
        wo=np.asarray(inputs["Wo"], bf),
        bd1=b2(inputs["bd1"], 4),
        bd2=b2(inputs["bd2"], 2),
        bdf=b2(inputs["bdf"], 1),
        bt1=b2(inputs["bt1"], 8),
        bt2=b2(inputs["bt2"], 8) * 16,
        bt3=b2(inputs["bt3"], 4),
        bt4=b2(inputs["bt4"], 2) * 16,
        bo=np.asarray(inputs["bo"], np.float32).reshape(1, 1),
    )

    in_maps = []
    for c in range(N_CORES):
        sl = slice(c * cfg.bc, (c + 1) * cfg.bc)
        ci = cat[sl].astype(np.int64)
        rows = (np.arange(N_CAT, dtype=np.int64) * cfg.vocab)[None, :] + ci
        a = np.zeros((cfg.nblk, P, N_FEAT), dtype=np.int64)
        a[:, :, :N_CAT] = rows.reshape(cfg.nblk, P, N_CAT)
        idxc = np.ascontiguousarray(
            a.reshape(cfg.nblk, N_FEAT * P)
            .reshape(cfg.nblk, N_FEAT, P)
            .transpose(2, 0, 1)
            .reshape(P, cfg.nblk * N_FEAT)
        ).astype(np.int32)
        xtc = np.ascontiguousarray(dx[sl].T.astype(bf))  # [13, bc]
        in_maps.append(dict(shared, idx=idxc, xt=xtc))
    return in_maps


def run_cores(inputs, cfg: Cfg, trace=False, **kw):
    import concourse.bass_utils as bass_utils

    nc = _get_nc(cfg)
    in_maps = _prep_host(inputs, cfg)
    res = bass_utils.run_bass_kernel_spmd(
        nc, in_maps, core_ids=list(range(N_CORES)), trace=trace, **kw
    )
    out = np.concatenate([r["out"] for r in res.results], axis=0)
    return out, res


def kernel(**inputs) -> np.ndarray:
    cfg = Cfg()
    out, _ = run_cores(inputs, cfg)
    return out.astype(np.float32)



# revision 38
# speedup vs baseline: 1.1184x; 1.1184x over previous
"""DLRM on 8 Trainium2 NeuronCores, data-parallel over the batch.  v7.

v7 = staged phase-I pipeline, two btiles deep (trace-driven):
  - phase I is emitted as five independent stages (gather / zt-transpose /
    gram / extract / x1-transpose) so no engine queue waits on work that
    was produced later than its own next item (head-of-line blocking).
  - btile v's MLP overlaps phase I of btile v+2 (x1ta triple-buffered);
    the copies->extract->x1t tail gets a ~2-btile window, absorbing the
    8-sem-lane round-robin coupling between unrelated DMAs.
  - extracts all on the scalar ring: on gpsimd they poison the per-btile
    SWDGE DRAIN barrier (9.9us stalls); sync stays transposes-only.
  - single wide DMA per weight matrix (3D AP over k-tiles), issued on
    sync at kernel start; w1f host-padded to 768 rows.
  - x1 transpose lands directly in its x1ta slot (strided 3D dst AP).
Earlier: MLP ReLU+bias on DVE (tensor_scalar add+max); bf16 table;
both xbar transposes serialized on sync (concurrent DMA-transposes on
the two HWDGE rings corrupt data); ACT/DVE alternate the PSUM copies.
"""

import numpy as np
import ml_dtypes

import concourse.bass as bass
import concourse.mybir as mybir
import concourse.tile as tile
from concourse import bacc
from concourse.bass import ds, ts

F32 = mybir.dt.float32
BF16 = mybir.dt.bfloat16
F8 = mybir.dt.float8e4
I32 = mybir.dt.int32
DR = mybir.MatmulPerfMode.DoubleRow

N_CAT = 26
N_FEAT = 27          # 26 embeddings + dense
E = 128
P = 128
N_CORES = 8

AFT = mybir.ActivationFunctionType


class Cfg:
    def __init__(self, vocab=100000, nblk=16):
        self.vocab = vocab
        self.nblk = nblk                  # 128-sample blocks per core
        self.bc = nblk * P                # batch per core
        self.vblk = min(4, nblk)          # blocks per btile
        self.vt = self.vblk * P           # btile size (free dim N)
        self.nv = nblk // self.vblk       # btiles per core


def build_kernel(nc: bass.Bass, tc: tile.TileContext, cfg: Cfg):
    from contextlib import ExitStack
    with ExitStack() as ctx:
        _build_kernel(nc, tc, cfg, ctx)


def _build_kernel(nc: bass.Bass, tc: tile.TileContext, cfg: Cfg, ctx):
    NBLK, BC, VT, NV, VBLK = cfg.nblk, cfg.bc, cfg.vt, cfg.nv, cfg.vblk

    # ---------------- DRAM I/O ----------------
    emb = nc.dram_tensor("emb", [N_CAT * cfg.vocab, E], BF16, kind="ExternalInput").ap()
    # idx[p, t*27 + c] = emb row for block t, flat n = c*128 + p, n = s*27 + j
    idx_d = nc.dram_tensor("idx", [P, NBLK * N_FEAT], I32, kind="ExternalInput").ap()
    xt_d = nc.dram_tensor("xt", [13, BC], BF16, kind="ExternalInput").ap()

    def win(name, shape):
        return nc.dram_tensor(name, shape, BF16, kind="ExternalInput").ap()

    wd1_d = win("wd1", [13, 512])
    wd2_d = win("wd2", [512, 256])
    wdf_d = win("wdf", [256, 128])
    w1f_d = win("w1f", [768, 1024])
    w1d_d = win("w1d", [128, 1024])
    wt2_d = win("wt2", [1024, 1024])
    wt3_d = win("wt3", [1024, 512])
    wt4_d = win("wt4", [512, 256])
    wo_d = win("wo", [256, 1])

    def bin_(name, shape):
        return nc.dram_tensor(name, shape, F32, kind="ExternalInput").ap()

    bd1_d = bin_("bd1", [P, 4])
    bd2_d = bin_("bd2", [P, 2])
    bdf_d = bin_("bdf", [P, 1])
    bt1_d = bin_("bt1", [P, 8])
    bt2_d = bin_("bt2", [P, 8])
    bt3_d = bin_("bt3", [P, 4])
    bt4_d = bin_("bt4", [P, 2])
    bo_d = bin_("bo", [1, 1])

    out_d = nc.dram_tensor("out", [BC, 1], F32, kind="ExternalOutput").ap()

    x1d = [
        nc.dram_tensor(f"x1scratch{t}", [P, 768], BF16, kind="Internal").ap()
        for t in range(NBLK)
    ]

    # ---------------- pools ----------------
    const = ctx.enter_context(tc.tile_pool(name="const", bufs=1))
    gpool = ctx.enter_context(tc.tile_pool(name="gather", bufs=4))
    zpool = ctx.enter_context(tc.tile_pool(name="zt", bufs=3))
    spool = ctx.enter_context(tc.tile_pool(name="spack", bufs=3))
    bpool = ctx.enter_context(tc.tile_pool(name="x1b", bufs=2))
    xtpool = ctx.enter_context(tc.tile_pool(name="x1t", bufs=3))
    opool = ctx.enter_context(tc.tile_pool(name="acts", bufs=1))
    p_s = ctx.enter_context(tc.tile_pool(name="pint", bufs=2, space="PSUM"))
    p_m = ctx.enter_context(tc.tile_pool(name="pmlp", bufs=3, space="PSUM"))


    ALU = mybir.AluOpType

    def relu_b(out, pm, bias_ap):
        nc.vector.tensor_scalar(
            out=out, in0=pm, scalar1=bias_ap, scalar2=0.0,
            op0=ALU.add, op1=ALU.max,
        )

    # ---------------- load constants ----------------
    # startup-critical first (bottom MLP inputs), big weights in single
    # wide DMAs on the sync ring (cheap descriptor gen, early issue)
    idx_sb = const.tile([P, NBLK * N_FEAT], I32)
    nc.scalar.dma_start(out=idx_sb[:], in_=idx_d)
    xtb = const.tile([13, BC], BF16, name="xtb")
    nc.scalar.dma_start(out=xtb[:], in_=xt_d)

    def load_w(name, d, k, n, eng):
        """One DMA for the whole weight; returns per-128-row k-tile views."""
        if k <= P:
            t = const.tile([k, n], BF16, name=name)
            eng.dma_start(out=t[:], in_=d[0:k, :])
            return [t[:]]
        T = d.shape[0] // P
        big = const.tile([P, T * n], BF16, name=name)
        eng.dma_start(
            out=big[:].rearrange("p (t n) -> p t n", t=T),
            in_=d.rearrange("(t p) n -> p t n", p=P),
        )
        views = []
        for i in range(T):
            ck = min(P, k - i * P)
            if ck <= 0:
                break
            views.append(big[ds(0, ck), ds(i * n, n)])
        return views

    def load_b(name, d, nm):
        t = const.tile([d.shape[0], nm], F32, name=name)
        nc.scalar.dma_start(out=t[:], in_=d)
        return t

    wd1 = load_w("wd1", wd1_d, 13, 512, nc.scalar)
    wd2 = load_w("wd2", wd2_d, 512, 256, nc.scalar)
    wdf = load_w("wdf", wdf_d, 256, 128, nc.scalar)
    w1f = load_w("w1f", w1f_d, 729, 1024, nc.sync)
    w1d = load_w("w1d", w1d_d, 128, 1024, nc.sync)
    wt2 = load_w("wt2", wt2_d, 1024, 1024, nc.sync)
    wt3 = load_w("wt3", wt3_d, 1024, 512, nc.sync)
    wt4 = load_w("wt4", wt4_d, 512, 256, nc.sync)
    wo = load_w("wo", wo_d, 256, 1, nc.sync)
    bd1 = load_b("bd1", bd1_d, 4)
    bd2 = load_b("bd2", bd2_d, 2)
    bdf = load_b("bdf", bdf_d, 1)
    bt1 = load_b("bt1", bt1_d, 8)
    bt2 = load_b("bt2", bt2_d, 8)
    bt3 = load_b("bt3", bt3_d, 4)
    bt4 = load_b("bt4", bt4_d, 2)
    bo = load_b("bo", bo_d, 1)

    # ---------------- bottom MLP (whole core batch) ----------------
    h1 = [const.tile([P, BC], BF16, name=f"h1_{m}") for m in range(4)]
    for m in range(4):
        for v in range(NV):
            pm = p_m.tile([P, VT], F32)
            nc.tensor.matmul(
                pm[:], wd1[0][:, ts(m, P)], xtb[:, ts(v, VT)],
                start=True, stop=True,
            )
            relu_b(h1[m][:, ts(v, VT)], pm[:], bd1[:, m : m + 1])

    h2 = [const.tile([P, BC], BF16, name=f"h2_{m}") for m in range(2)]
    for m in range(2):
        for v in range(NV):
            pm = p_m.tile([P, VT], F32)
            for k in range(4):
                nc.tensor.matmul(
                    pm[:], wd2[k][:, ts(m, P)], h1[k][:, ts(v, VT)],
                    start=(k == 0), stop=(k == 3),
                )
            relu_b(h2[m][:, ts(v, VT)], pm[:], bd2[:, m : m + 1])

    denseT = const.tile([P, BC], BF16)
    for v in range(NV):
        pm = p_m.tile([P, VT], F32)
        for k in range(2):
            nc.tensor.matmul(
                pm[:], wdf[k][:], h2[k][:, ts(v, VT)],
                start=(k == 0), stop=(k == 1),
            )
        relu_b(denseT[:, ts(v, VT)], pm[:], bdf[:, 0:1])

    # ---------------- phase I, split into independently-emitted stages --
    # Per-engine queues execute in (priority = emission) order, so a stage
    # that waits on cross-engine work must NOT be emitted ahead of work the
    # same engine needs soon (head-of-line blocking).  Stages for block t:
    #   gather(t)   gpsimd   indirect gather          -> g[t]
    #   zt(t)       sync     xbar transpose + ztd     -> zt[t]
    #   gram(t)     PE       32 Gram MMs + 8 copies   -> spk[t]
    #   ext(t)      scalar/gpsimd  4 pair-extract DMAs -> x1d[t] (DRAM)
    #   x1t(t)      sync     xbar transpose           -> x1ta slot
    x1tas = {}
    g_t, zt_t, spk_t = {}, {}, {}

    def get_x1ta(v):
        if v not in x1tas:
            x1tas[v] = xtpool.tile([P, 6 * VBLK * P], BF16, name="x1ta")
        return x1tas[v]

    def stage_gather(t):
        if t >= NBLK:
            return
        g = gpool.tile([P, N_FEAT * E], BF16)
        g_t[t] = g
        nc.gpsimd.indirect_dma_start(
            out=g[:],
            out_offset=None,
            in_=emb,
            in_offset=bass.IndirectOffsetOnAxis(
                ap=idx_sb[:, t * N_FEAT : (t + 1) * N_FEAT], axis=0
            ),
        )

    def stage_zt(t):
        if t >= NBLK:
            return
        # Both xbar transposes stay on ONE ring: concurrent DMA-transposes
        # on the two HWDGE rings corrupt data (xbar hazard; Tile does not
        # guard it). Serializing them on sync keeps results correct.
        zt = zpool.tile([P, N_FEAT * E], BF16)
        zt_t[t] = zt
        nc.sync.dma_start(
            out=zt[:].rearrange("p (c s) -> p c s", c=N_FEAT),
            in_=g_t.pop(t)[:],
            transpose=True,
        )
        ztd = zt[:].rearrange("p (s j) -> p s j", j=N_FEAT)[:, :, N_CAT]
        nc.vector.tensor_copy(out=ztd, in_=denseT[:, ts(t, P)])

    def stage_gram(t):
        if t >= NBLK:
            return
        zt = zt_t.pop(t)
        spk = spool.tile([108, 32 * 108], BF16)
        spk_t[t] = spk
        for q0 in range(0, 32, 4):
            pi = p_s.tile([108, 4 * 108], F32)
            for q in range(q0, q0 + 4):
                op = zt[:, ds(q * 108, 108)]
                nc.tensor.matmul(
                    pi[:, ts(q - q0, 108)], op, op, start=True, stop=True
                )
            if (q0 // 4) % 2 == 0:
                nc.scalar.activation(
                    spk[:, ds(q0 * 108, 4 * 108)], pi[:], AFT.Copy
                )
            else:
                nc.vector.tensor_copy(
                    out=spk[:, ds(q0 * 108, 4 * 108)], in_=pi[:]
                )

    def stage_ext(t):
        if t >= NBLK:
            return
        spk = spk_t.pop(t)
        for sj in range(4):
            src = spk[ds(sj * 27, 27), :].rearrange(
                "p (q s2 j2) -> p q s2 j2", s2=4, j2=27
            )[:, :, sj, :]
            dstv = x1d[t][:, :729].rearrange(
                "(q s) (j1 j2) -> s j1 q j2", s=4, j2=27
            )[sj, :, :, :]
            nc.scalar.dma_start(out=dstv, in_=src)

    def stage_x1t(t):
        if t >= NBLK:
            return
        v, tt = t // VBLK, t % VBLK
        x1v = get_x1ta(v)[:].rearrange("p (k b s) -> p k b s", k=6, s=P)
        nc.sync.dma_start(
            out=x1v[:, :, tt, :],
            in_=x1d[t], transpose=True,
        )

    # ---------------- main: prologue + software-pipelined btiles --------
    # Two-btile-deep pipeline: btile v's MLP runs while btile v+2's blocks
    # move through the phase-I stages; every stage gets a ~2-btile window.
    NPRO = min(2 * VBLK, NBLK)
    for t in range(NPRO):
        stage_gather(t)
    for t in range(NPRO):
        stage_zt(t)
        stage_gram(t)
        if t >= 2:
            stage_ext(t - 2)
    for t in range(max(0, NPRO - 2), NPRO):
        stage_ext(t)
    for t in range(NPRO):
        stage_x1t(t)

    for v in range(NV):
        x1ta = get_x1ta(v)
        x1t = [x1ta[:, ds(k * VT, VT)] for k in range(6)]
        b = (v + 2) * VBLK  # btile v+2's blocks, prepared during this one

        # layer 1: K = 729 pairs (+pad, zero weights) + 128 dense
        o1 = opool.tile([P, 8 * VT], BF16, name="o1")
        for m in range(8):
            pm = p_m.tile([P, VT], F32)
            for k in range(6):
                nc.tensor.matmul(
                    pm[:], w1f[k][:, ts(m, P)], x1t[k][: w1f[k].shape[0], :],
                    start=(k == 0), stop=False,
                )
            nc.tensor.matmul(
                pm[:], w1d[0][:, ts(m, P)], denseT[:, ts(v, VT)],
                start=False, stop=True,
            )
            relu_b(o1[:, ts(m, VT)], pm[:], bt1[:, m : m + 1])
            if m in (1, 3, 5, 7):
                stage_gather(b + m // 2)
        x1tas.pop(v)

        o2 = opool.tile([P, 8 * VT], BF16, name="o2")
        for m in range(8):
            pm = p_m.tile([P, VT], F32)
            for k in range(8):
                nc.tensor.matmul(
                    pm[:], wt2[k][:, ts(m, P)], o1[:, ts(k, VT)],
                    start=(k == 0), stop=(k == 7),
                )
            relu_b(o2[:, ts(m, VT)], pm[:], bt2[:, m : m + 1])
            if m % 2 == 0:
                stage_zt(b + m // 2)
                if m >= 2:
                    stage_ext(b + m // 2 - 1)
            else:
                stage_gram(b + m // 2)

        o3 = opool.tile([P, 4 * VT], BF16, name="o3")
        for m in range(4):
            pm = p_m.tile([P, VT], F32)
            for k in range(8):
                nc.tensor.matmul(
                    pm[:], wt3[k][:, ts(m, P)], o2[:, ts(k, VT)],
                    start=(k == 0), stop=(k == 7),
                )
            relu_b(o3[:, ts(m, VT)], pm[:], bt3[:, m : m + 1])
            if m == 0:
                stage_ext(b + 3)
            else:
                stage_x1t(b + m - 1)

        o4 = opool.tile([P, 2 * VT], BF16, name="o4")
        for m in range(2):
            pm = p_m.tile([P, VT], F32)
            for k in range(4):
                nc.tensor.matmul(
                    pm[:], wt4[k][:, ts(m, P)], o3[:, ts(k, VT)],
                    start=(k == 0), stop=(k == 3),
                )
            relu_b(o4[:, ts(m, VT)], pm[:], bt4[:, m : m + 1])
            if m == 0:
                stage_x1t(b + 3)

        pm = p_m.tile([1, VT], F32)
        for k in range(2):
            nc.tensor.matmul(
                pm[:], wo[k][:], o4[:, ts(k, VT)],
                start=(k == 0), stop=(k == 1),
            )
        zf = bpool.tile([1, VT], F32, name="zfinal")
        nc.scalar.activation(zf[:], pm[:], AFT.Sigmoid, bias=bo[:])
        nc.scalar.dma_start(
            out=out_d[ts(v, VT), :].rearrange("b one -> one b"), in_=zf[:]
        )


# ---------------------------------------------------------------------------
# host side
# ---------------------------------------------------------------------------

_CACHE = {}


def _get_nc(cfg: Cfg):
    key = (cfg.vocab, cfg.nblk)
    if key in _CACHE:
        return _CACHE[key]
    nc = bacc.Bacc(
        "TRN2",
        target_bir_lowering=False,
        debug=False,
        enable_asserts=False,
        num_devices=N_CORES,
    )
    with tile.TileContext(nc) as tc:
        build_kernel(nc, tc, cfg)
    nc.compile()
    _CACHE[key] = nc
    return nc


def _prep_host(inputs, cfg: Cfg):
    """Build the per-core in_maps from full inputs."""
    bf = ml_dtypes.bfloat16
    emb = np.ascontiguousarray(
        np.asarray(inputs["emb_table"], dtype=np.float32)
        .reshape(N_CAT * cfg.vocab, E)
        .astype(bf)
    )
    cat = np.asarray(inputs["cat_idx"])
    dx = np.asarray(inputs["dense_x"], dtype=np.float32)

    iu, ju = np.triu_indices(N_FEAT, k=1)
    wt1 = np.asarray(inputs["Wt1"], dtype=np.float32)  # [479, 1024]
    w1f = np.zeros((768, 1024), dtype=np.float32)  # 729 pairs padded to 6*128
    w1fv = w1f[:729].reshape(N_FEAT, N_FEAT, 1024)
    w1fv[iu, ju] = 0.5 * wt1[: len(iu)]
    w1fv[ju, iu] = 0.5 * wt1[: len(iu)]
    w1d = wt1[len(iu) :]  # [128, 1024]

    def b2(x, nm):  # bias [N] -> [128, nm]
        return np.ascontiguousarray(
            np.asarray(x, np.float32).reshape(nm, P).T
        )

    shared = dict(
        emb=emb,
        wd1=np.asarray(inputs["Wd1"], bf),
        wd2=np.asarray(inputs["Wd2"], bf),
        wdf=np.asarray(inputs["Wdf"], bf),
        w1f=np.asarray(w1f, bf),
        w1d=np.ascontiguousarray(np.asarray(w1d, bf)),
        wt2=np.asarray(inputs["Wt2"], bf),
        wt3=np.asarray(inputs["Wt3"], bf),
        wt4=np.asarray(inputs["Wt4"], bf),
        wo=np.asarray(inputs["Wo"], bf),
        bd1=b2(inputs["bd1"], 4),
        bd2=b2(inputs["bd2"], 2),
        bdf=b2(inputs["bdf"], 1),
        bt1=b2(inputs["bt1"], 8),
        bt2=b2(inputs["bt2"], 8),
        bt3=b2(inputs["bt3"], 4),
        bt4=b2(inputs["bt4"], 2),
        bo=np.asarray(inputs["bo"], np.float32).reshape(1, 1),
    )

    in_maps = []
    for c in range(N_CORES):
        sl = slice(c * cfg.bc, (c + 1) * cfg.bc)
        ci = cat[sl].astype(np.int64)
        rows = (np.arange(N_CAT, dtype=np.int64) * cfg.vocab)[None, :] + ci
        a = np.zeros((cfg.nblk, P, N_FEAT), dtype=np.int64)
        a[:, :, :N_CAT] = rows.reshape(cfg.nblk, P, N_CAT)
        idxc = np.ascontiguousarray(
            a.reshape(cfg.nblk, N_FEAT * P)
            .reshape(cfg.nblk, N_FEAT, P)
            .transpose(2, 0, 1)
            .reshape(P, cfg.nblk * N_FEAT)
        ).astype(np.int32)
        xtc = np.ascontiguousarray(dx[sl].T.astype(bf))  # [13, bc]
        in_maps.append(dict(shared, idx=idxc, xt=xtc))
    return in_maps


def run_cores(inputs, cfg: Cfg, trace=False, **kw):
    import concourse.bass_utils as bass_utils

    nc = _get_nc(cfg)
    in_maps = _prep_host(inputs, cfg)
    res = bass_utils.run_bass_kernel_spmd(
        nc, in_maps, core_ids=list(range(N_CORES)), trace=trace, **kw
    )
    out = np.concatenate([r["out"] for r in res.results], axis=0)
    return out, res


def kernel(**inputs) -> np.ndarray:
    cfg = Cfg()
    out, _ = run_cores(inputs, cfg)
    return out.astype(np.float32)

